# revision 4
# baseline (speedup 1.0000x reference)
"""Trainium2 Bass kernel for nn_ExtendableSheafGCNLayer (8-core SPMD).

Sharding: edges are split contiguously across the 8 NeuronCores. Host-side
preprocessing touches only the *index* data (edge_index): each core's edge
slice is sorted by destination node u, node-segments are packed into
256-slot groups (a segment never spans a group), and per-slot index tables
are emitted. The two per-edge transform tensors are interleaved row-wise on
the host into one [E/8, 512] array so a single 2 KB indirect-DMA descriptor
fetches both matrices of an edge.

Per core, per 128-slot tile the device kernel:
  - indirect-DMA gathers AB rows (2 KB) by sorted edge id, x = embeddings[v]
    (64 B) and w = adj[v, u] (4 B) via [128,1] offset tables
  - ScalarE computes xw = w * x  (h_v is linear in x, so the adjacency
    weight is folded into the gathered embedding)
  - VectorE: h_mid = A_vu @ xw (broadcast multiply + segmented reduce),
    h_v = A_uv^T @ h_mid (strided view + reduce), onehot[p,j] = (j == seg[p])
  - TensorE: psum[j,:] += onehot^T @ h_v accumulates per-segment messages
    across the group's tiles (fp32 matmul)
  - per group the [64, 16] result is scattered to the per-core node table

Host sums the 8 per-core tables (the cross-device reduce of the hint).
"""
import numpy as np

N = 10000
E = 500000
D = 16
N_CORES = 8
TILE = 128
TPG = 2                   # tiles per group
GS = TILE * TPG           # slots per group
J = 64                    # max segments (psum rows) per group
E_PER_CORE = E // N_CORES
REPLICATED = ("emb", "adj", "iota")


def _host_prep(edge_index):
    u_all = np.asarray(edge_index[0], dtype=np.int64)
    v_all = np.asarray(edge_index[1], dtype=np.int64)

    cores = []
    for c in range(N_CORES):
        lo, hi = c * E_PER_CORE, (c + 1) * E_PER_CORE
        u = u_all[lo:hi]
        v = v_all[lo:hi]
        order = np.argsort(u, kind="stable").astype(np.int64)
        us = u[order]
        bound = np.flatnonzero(np.diff(us)) + 1
        starts = np.concatenate(([0], bound))
        ends = np.concatenate((bound, [len(us)]))
        seg_lens = (ends - starts).astype(np.int64)
        seg_u = us[starts]
        assert seg_lens.max() <= GS, "segment larger than a group"

        groups = []
        cur, cur_slots = [], 0
        for si in range(len(seg_lens)):
            L = int(seg_lens[si])
            if cur_slots + L > GS or len(cur) >= J:
                groups.append(cur)
                cur, cur_slots = [], 0
            cur.append(si)
            cur_slots += L
        if cur:
            groups.append(cur)

        n_groups = len(groups)
        S = n_groups * GS
        eid = np.zeros(S, dtype=np.int32)
        vv = np.zeros(S, dtype=np.int32)
        wlin = np.zeros(S, dtype=np.int32)
        seg = np.full(S, -1.0, dtype=np.float32)
        dest = np.zeros((n_groups, J), dtype=np.int32)
        for g, segs in enumerate(groups):
            base = g * GS
            off = 0
            for j, si in enumerate(segs):
                s0, s1 = int(starts[si]), int(ends[si])
                L = s1 - s0
                sl = slice(base + off, base + off + L)
                le = order[s0:s1]
                eid[sl] = le
                vv[sl] = v[le]
                wlin[sl] = v[le] * N + seg_u[si]
                seg[sl] = j
                dest[g, j] = seg_u[si]
                off += L
            dest[g, len(segs):] = N + np.arange(J - len(segs))
        cores.append(dict(eid=eid, v=vv, wlin=wlin, seg=seg, dest=dest,
                          n_groups=n_groups))

    n_groups_max = max(cd["n_groups"] for cd in cores)
    for cd in cores:
        ng = cd["n_groups"]
        if ng < n_groups_max:
            pad_g = n_groups_max - ng
            pad_s = pad_g * GS
            cd["eid"] = np.concatenate([cd["eid"], np.zeros(pad_s, np.int32)])
            cd["v"] = np.concatenate([cd["v"], np.zeros(pad_s, np.int32)])
            cd["wlin"] = np.concatenate([cd["wlin"], np.zeros(pad_s, np.int32)])
            cd["seg"] = np.concatenate([cd["seg"], np.full(pad_s, -1.0, np.float32)])
            pad_dest = N + np.tile(np.arange(J, dtype=np.int32), (pad_g, 1))
            cd["dest"] = np.concatenate([cd["dest"], pad_dest], axis=0)
            cd["n_groups"] = n_groups_max
    return cores, n_groups_max


_BUILD_CACHE = {}


def _build(n_groups):
    if n_groups in _BUILD_CACHE:
        return _BUILD_CACHE[n_groups]
    import concourse.bass as bass
    import concourse.bacc as bacc
    import concourse.mybir as mybir
    import concourse.tile as tile

    f32 = mybir.dt.float32
    i32 = mybir.dt.int32
    NT = n_groups * TPG
    NTAB = N + J

    nc = bacc.Bacc(None, target_bir_lowering=False, debug=False)
    d_AB = nc.declare_dram_parameter("AB", [E_PER_CORE, 2 * D * D], f32, isOutput=False)
    d_emb = nc.declare_dram_parameter("emb", [N, D], f32, isOutput=False)
    d_adj = nc.declare_dram_parameter("adj", [N * N, 1], f32, isOutput=False)
    d_eid = nc.declare_dram_parameter("eid", [TILE, NT], i32, isOutput=False)
    d_v = nc.declare_dram_parameter("v", [TILE, NT], i32, isOutput=False)
    d_wlin = nc.declare_dram_parameter("wlin", [TILE, NT], i32, isOutput=False)
    d_seg = nc.declare_dram_parameter("seg", [TILE, NT], f32, isOutput=False)
    d_dest = nc.declare_dram_parameter("dest", [J, n_groups], i32, isOutput=False)
    d_iota = nc.declare_dram_parameter("iota", [TILE, J], f32, isOutput=False)
    d_out = nc.declare_dram_parameter("out", [NTAB, D], f32, isOutput=True)

    with tile.TileContext(nc) as tc:
        with (
            tc.tile_pool(name="idx", bufs=1) as idxp,
            tc.tile_pool(name="ab", bufs=6) as abp,
            tc.tile_pool(name="xw", bufs=6) as xwp,
            tc.tile_pool(name="work", bufs=3) as wkp,
            tc.tile_pool(name="small", bufs=4) as smp,
            tc.tile_pool(name="psum", bufs=4, space="PSUM") as psump,
            tc.tile_pool(name="outp", bufs=4) as outp,
        ):
            t_eid = idxp.tile([TILE, NT], i32)
            t_v = idxp.tile([TILE, NT], i32)
            t_wlin = idxp.tile([TILE, NT], i32)
            t_seg = idxp.tile([TILE, NT], f32)
            t_dest = idxp.tile([J, n_groups], i32)
            t_iota = idxp.tile([TILE, J], f32)
            nc.gpsimd.dma_start(out=t_eid[:], in_=d_eid[:, :])
            nc.gpsimd.dma_start(out=t_v[:], in_=d_v[:, :])
            nc.gpsimd.dma_start(out=t_wlin[:], in_=d_wlin[:, :])
            nc.gpsimd.dma_start(out=t_seg[:], in_=d_seg[:, :])
            nc.gpsimd.dma_start(out=t_dest[:], in_=d_dest[:, :])
            nc.gpsimd.dma_start(out=t_iota[:], in_=d_iota[:, :])

            for g in range(n_groups):
                psum_m = psump.tile([J, D], f32, tag="pm")
                for ti in range(TPG):
                    t = g * TPG + ti
                    ab = abp.tile([TILE, 2 * D * D], f32, tag="ab")
                    x = xwp.tile([TILE, D], f32, tag="x")
                    w = xwp.tile([TILE, 1], f32, tag="w")
                    xw = xwp.tile([TILE, D], f32, tag="xwv")
                    nc.gpsimd.indirect_dma_start(
                        out=ab[:], out_offset=None, in_=d_AB[:],
                        in_offset=bass.IndirectOffsetOnAxis(
                            ap=t_eid[:, t:t + 1], axis=0))
                    nc.gpsimd.indirect_dma_start(
                        out=x[:], out_offset=None, in_=d_emb[:],
                        in_offset=bass.IndirectOffsetOnAxis(
                            ap=t_v[:, t:t + 1], axis=0))
                    nc.gpsimd.indirect_dma_start(
                        out=w[:], out_offset=None, in_=d_adj[:],
                        in_offset=bass.IndirectOffsetOnAxis(
                            ap=t_wlin[:, t:t + 1], axis=0))
                    # xw = w * x on ScalarE (adjacency weight folded into x)
                    nc.scalar.mul(out=xw[:], in_=x[:], mul=w[:, 0:1])

                    a_vu = ab[:, 0:D * D]
                    a_uv = ab[:, D * D:2 * D * D]
                    tmp = wkp.tile([TILE, D * D], f32, tag="tmp")
                    h_mid = smp.tile([TILE, D], f32, tag="hmid")
                    tmp2 = wkp.tile([TILE, D * D], f32, tag="tmp2")
                    h_v = smp.tile([TILE, D], f32, tag="hv")
                    oh = smp.tile([TILE, J], f32, tag="oh")

                    xb = xw[:].unsqueeze(1).broadcast_to([TILE, D, D])
                    nc.vector.tensor_tensor(
                        out=tmp[:], in0=a_vu.rearrange("p (y x) -> p y x", x=D),
                        in1=xb, op=mybir.AluOpType.mult)
                    nc.vector.tensor_reduce(
                        out=h_mid[:], in_=tmp[:].rearrange("p (y x) -> p y x", x=D),
                        axis=mybir.AxisListType.X, op=mybir.AluOpType.add)
                    auv_xmaj = a_uv.rearrange("p (y x) -> p x y", x=D)
                    hb = h_mid[:].unsqueeze(1).broadcast_to([TILE, D, D])
                    nc.vector.tensor_tensor(
                        out=tmp2[:].rearrange("p (x y) -> p x y", y=D),
                        in0=auv_xmaj, in1=hb, op=mybir.AluOpType.mult)
                    nc.vector.tensor_reduce(
                        out=h_v[:], in_=tmp2[:].rearrange("p (x y) -> p x y", y=D),
                        axis=mybir.AxisListType.X, op=mybir.AluOpType.add)
                    nc.vector.tensor_scalar(
                        out=oh[:], in0=t_iota[:], scalar1=t_seg[:, t:t + 1],
                        scalar2=None, op0=mybir.AluOpType.is_equal)
                    nc.tensor.matmul(
                        out=psum_m[:], lhsT=oh[:], rhs=h_v[:],
                        start=(ti == 0), stop=(ti == TPG - 1))
                m_sb = outp.tile([J, D], f32, tag="msb")
                nc.scalar.copy(out=m_sb[:], in_=psum_m[:])
                nc.gpsimd.indirect_dma_start(
                    out=d_out[:], out_offset=bass.IndirectOffsetOnAxis(
                        ap=t_dest[:, g:g + 1], axis=0),
                    in_=m_sb[:], in_offset=None)
    nc.compile()
    _BUILD_CACHE[n_groups] = nc
    return nc


def _make_in_maps(adj_matrix, embeddings, A_uv, A_vu, cores):
    iota = np.ascontiguousarray(
        np.broadcast_to(np.arange(J, dtype=np.float32), (TILE, J)))
    adj_flat = np.asarray(adj_matrix, dtype=np.float32).reshape(N * N, 1)
    emb = np.asarray(embeddings, dtype=np.float32)
    A_uv = np.asarray(A_uv, dtype=np.float32).reshape(E, D * D)
    A_vu = np.asarray(A_vu, dtype=np.float32).reshape(E, D * D)
    in_maps = []
    for c in range(N_CORES):
        lo, hi = c * E_PER_CORE, (c + 1) * E_PER_CORE
        cd = cores[c]
        AB = np.concatenate([A_vu[lo:hi], A_uv[lo:hi]], axis=1)
        in_maps.append({
            "AB": AB,
            "emb": emb,
            "adj": adj_flat,
            "eid": np.ascontiguousarray(cd["eid"].reshape(-1, TILE).T),
            "v": np.ascontiguousarray(cd["v"].reshape(-1, TILE).T),
            "wlin": np.ascontiguousarray(cd["wlin"].reshape(-1, TILE).T),
            "seg": np.ascontiguousarray(cd["seg"].reshape(-1, TILE).T),
            "dest": np.ascontiguousarray(cd["dest"].T),
            "iota": iota,
        })
    return in_maps


def _run(nc, in_maps, reps=1):
    """Sharded PJRT runner. adj/emb/iota are replicated (uploaded once);
    everything else is sharded across the 8 cores. Returns (per-core
    outputs of the first execution, list of per-exec wall times)."""
    import time
    import jax
    import concourse.mybir as mybir
    from concourse.bass2jax import (
        _bass_exec_p, install_neuronx_cc_hook, partition_id_tensor)
    from jax.sharding import Mesh, PartitionSpec, NamedSharding
    from jax.experimental.shard_map import shard_map

    install_neuronx_cc_hook()
    partition_name = nc.partition_id_tensor.name if nc.partition_id_tensor else None
    in_names, out_names, out_avals = [], [], []
    for alloc in nc.m.functions[0].allocations:
        if not isinstance(alloc, mybir.MemoryLocationSet):
            continue
        name = alloc.memorylocations[0].name
        if alloc.kind == "ExternalInput":
            if name != partition_name:
                in_names.append(name)
        elif alloc.kind == "ExternalOutput":
            out_names.append(name)
            out_avals.append(jax.core.ShapedArray(
                tuple(alloc.tensor_shape), mybir.dt.np(alloc.dtype)))
    n_params = len(in_names)
    n_outs = len(out_names)
    all_in_names = in_names + out_names
    if partition_name is not None:
        all_in_names = all_in_names + [partition_name]

    def _body(*args):
        operands = list(args)
        if partition_name is not None:
            operands.append(partition_id_tensor())
        outs = _bass_exec_p.bind(
            *operands,
            out_avals=tuple(out_avals),
            in_names=tuple(all_in_names),
            out_names=tuple(out_names),
            lowering_input_output_aliases=(),
            sim_require_finite=True,
            sim_require_nnan=True,
            nc=nc,
        )
        return tuple(outs)

    devices = jax.devices()[:N_CORES]
    mesh = Mesh(np.asarray(devices), ("core",))
    shard = NamedSharding(mesh, PartitionSpec("core"))
    repl = NamedSharding(mesh, PartitionSpec())
    in_specs = tuple(
        PartitionSpec() if k in REPLICATED else PartitionSpec("core")
        for k in in_names) + (PartitionSpec("core"),) * n_outs
    out_specs = (PartitionSpec("core"),) * n_outs
    donate = tuple(range(n_params, n_params + n_outs))
    sharded = jax.jit(
        shard_map(_body, mesh=mesh, in_specs=in_specs, out_specs=out_specs,
                  check_rep=False),
        donate_argnums=donate, keep_unused=True)

    dev_in = []
    for i, k in enumerate(in_names):
        if k in REPLICATED:
            dev_in.append(jax.device_put(np.asarray(in_maps[0][k]), repl))
        else:
            arr = np.concatenate(
                [np.asarray(in_maps[c][k]) for c in range(N_CORES)], axis=0)
            dev_in.append(jax.device_put(arr, shard))
    jax.block_until_ready(dev_in)

    def zeros_set():
        return [jax.device_put(
            np.zeros((N_CORES * a.shape[0], *a.shape[1:]), a.dtype), shard)
            for a in out_avals]

    zsets = [zeros_set() for _ in range(reps)]
    jax.block_until_ready(zsets)

    first_out = None
    times = []
    import time as _t
    for r in range(reps):
        t0 = _t.perf_counter()
        outs = sharded(*dev_in, *zsets[r])
        jax.block_until_ready(outs)
        times.append(_t.perf_counter() - t0)
        if first_out is None:
            first_out = [np.asarray(o) for o in outs]

    results = [
        {name: first_out[i].reshape(N_CORES, *out_avals[i].shape)[c]
         for i, name in enumerate(out_names)}
        for c in range(N_CORES)
    ]
    return results, times


def kernel(adj_matrix, embeddings, A_uv, A_vu, edge_index, _reps=1,
           _return_times=False):
    cores, n_groups = _host_prep(edge_index)
    nc = _build(n_groups)
    in_maps = _make_in_maps(adj_matrix, embeddings, A_uv, A_vu, cores)
    results, times = _run(nc, in_maps, reps=_reps)
    out = np.zeros((N, D), dtype=np.float32)
    for c in range(N_CORES):
        out += results[c]["out"][:N]
    if _return_times:
        return out, times
    return out
